# revision 2
# baseline (speedup 1.0000x reference)
"""Trainium2 Bass kernel: 3-level db4 DWT front-end (analysis + per-band
single-band reconstructions).

Input  x : [16, 128, 8192] float32
Output   : [4, 16, 128, 8192] float32  (bands: approx, d3, d2, d1)

Sharding: depthwise per-(batch, channel) row -> flatten to 2048 independent
rows of length 8192; 256 rows per NeuronCore (8 cores), two [128, *]
partition tiles per core. No cross-core communication.

v2 layout (4-engine balance, f16 data path):
  - I/O in float16: host converts x -> f16 (4 MiB/core in), all four band
    outputs written f16 (16 MiB/core out), converted back to f32 on host.
  - DVE: analysis chains in f16 via even/odd split inputs: muls hit the
    4x DVE mode (packed 2-byte operands), adds hit 2x -> ~5.5 cyc per
    8-tap output column instead of 8 (STT has no fast modes).
  - PE: all synthesis as diag matmuls (f16, 1 cyc/row). The two trailing
    REC_LO stages of bands 0/1 and the hi+lo pair of band 2 are fused into
    single up-4 composites (22 taps over 4 phases) -> fewer PE cycles and
    fewer PSUM evacuations.
  - Act: PSUM evacuations (f32 PSUM -> f16 SBUF, interleaving strided
    dest) + x deinterleave.
  - Pool: a3 chain (mul+add pairs) + a1/a2 deinterleaves.
"""

import numpy as np

import concourse.bass as bass
import concourse.tile as tile
from concourse import bacc, mybir
from concourse.bass_utils import run_bass_kernel_spmd

F32 = mybir.dt.float32
F16 = mybir.dt.float16
MULT = mybir.AluOpType.mult
ADD = mybir.AluOpType.add
EQ = mybir.AluOpType.is_equal

DEC_LO = np.array([-0.0105974018, 0.0328830117, 0.0308413818, -0.1870348117,
                   -0.0279837694, 0.6308807679, 0.7148465706, 0.2303778133], np.float64)
DEC_HI = np.array([-0.2303778133, 0.7148465706, -0.6308807679, -0.0279837694,
                   0.1870348117, 0.0308413818, -0.0328830117, -0.0105974018], np.float64)
REC_LO = DEC_LO[::-1].copy()
REC_HI = DEC_HI[::-1].copy()

L0, L1, L2, L3 = 8192, 4100, 2054, 1031
N_CORES = 8
ROWS_PER_CORE = 256
TILES_PER_CORE = 2
PSUM_CHUNK = 512


def _synth_phase_taps(w, phase):
    """(src_offset, weight) pairs for one conv_transpose phase after crop 7."""
    if phase == 0:
        return [(b, w[7 - 2 * b]) for b in range(4)]
    return [(c, w[8 - 2 * c]) for c in range(1, 5)]


def _compose_u4(w1, w2):
    """Taps of S_{w2}(S_{w1}(src)) as an up-4 map: out[4k+r] = sum_t w*src[k+off]."""
    out = {r: {} for r in range(4)}
    for r in range(4):
        p2 = r & 1
        c = (r - p2) // 2
        for off2, w2v in _synth_phase_taps(w2, p2):
            t = c + off2
            p1 = t & 1
            q = (t - p1) // 2
            for off1, w1v in _synth_phase_taps(w1, p1):
                out[r][q + off1] = out[r].get(q + off1, 0.0) + w2v * w1v
    return {r: sorted(out[r].items()) for r in out}


TAPS_LL = _compose_u4(REC_LO, REC_LO)   # bands 0/1 trailing lo-lo stages
TAPS_HL = _compose_u4(REC_HI, REC_LO)   # band 2 hi-then-lo


class Ctx:
    def __init__(self, nc, pool, obpool, pspool):
        self.nc = nc
        self.pool = pool
        self.obpool = obpool
        self.pspool = pspool
        self.diag = {}

    def build_consts(self):
        nc = self.nc
        ones = self.pool.tile([128, 128], F32, tag="ones")
        nc.vector.memset(ones[:], 1.0)
        ident = self.pool.tile([128, 128], F32, tag="ident")
        nc.gpsimd.affine_select(ident[:], ones[:], [[1, 128]], EQ, 0.0,
                                base=0, channel_multiplier=-1)
        vals = set()
        for w in (REC_LO, REC_HI):
            for p in (0, 1):
                vals.update(v for _, v in _synth_phase_taps(w, p))
        for taps in (TAPS_LL, TAPS_HL):
            for r in range(4):
                vals.update(v for _, v in taps[r])
        for i, w in enumerate(sorted(vals)):
            d = self.pool.tile([128, 128], F16, tag=f"diag{i}")
            nc.vector.tensor_scalar_mul(d[:], ident[:], float(w))
            self.diag[float(w)] = d


def _interleave(*op_lists):
    n = max(len(l) for l in op_lists)
    for i in range(n):
        for l in op_lists:
            if i < len(l):
                l[i]()


def _ana_f16_thunks(ctx, xe, xo, out, w, No, tmp):
    """out[:, i] = sum_k w[k]*xp[:, 2i+k] using even/odd split sources.
    8 muls (4x mode) + 7 adds (2x mode) on DVE via a scratch tile."""
    nc = ctx.nc
    ops = []
    for m in range(4):
        for par, src in ((0, xe), (1, xo)):
            wv = float(w[2 * m + par])
            s = src[:, m:m + No]
            if m == 0 and par == 0:
                ops.append(lambda o=out, s=s, v=wv: nc.vector.tensor_scalar_mul(o, s, v))
            else:
                t = tmp[:, :No]
                ops.append(lambda t=t, s=s, v=wv: nc.vector.tensor_scalar_mul(t, s, v))
                ops.append(lambda o=out, t=t: nc.vector.tensor_tensor(o, o, t, ADD))
    return ops


def _emit_pool_ana(ctx, xp, out, w, No):
    """8-tap analysis chain on Pool via mul+add pairs (strided src is fine)."""
    nc = ctx.nc
    tmp = ctx.pool.tile([128, No], F32, tag="ptmp")
    acc = ctx.pool.tile([128, No], F32, tag="pacc")
    for k in range(8):
        src = xp[:, k:k + 2 * No - 1:2]
        if k == 0:
            nc.gpsimd.tensor_scalar_mul(acc[:, :No], src, float(w[0]))
        elif k < 7:
            nc.gpsimd.tensor_scalar_mul(tmp[:, :No], src, float(w[k]))
            nc.gpsimd.tensor_tensor(acc[:, :No], acc[:, :No], tmp[:, :No], ADD)
        else:
            nc.gpsimd.tensor_scalar_mul(tmp[:, :No], src, float(w[k]))
            nc.gpsimd.tensor_tensor(out, acc[:, :No], tmp[:, :No], ADD)


def _emit_synth_pe(ctx, x, dest, taps_by_phase, T, stride):
    """Synthesis via diag matmuls: for each phase r, dest[:, r::stride] =
    sum_(off,w) w * x[:, k+off]. H = T//stride output cols per phase."""
    nc = ctx.nc
    H = T // stride
    for r, taps in taps_by_phase:
        for c0 in range(0, H, PSUM_CHUNK):
            n = min(PSUM_CHUNK, H - c0)
            ps = ctx.pspool.tile([128, PSUM_CHUNK], F32, tag="ps")
            for i, (off, wv) in enumerate(taps):
                rhs = x[:, c0 + off:c0 + off + n]
                nc.tensor.matmul(ps[:, :n], ctx.diag[float(wv)][:], rhs,
                                 start=(i == 0), stop=(i == len(taps) - 1))
            s0 = r + stride * c0
            nc.scalar.copy(dest[:, s0:s0 + stride * (n - 1) + 1:stride], ps[:, :n])


def _emit_reflect(ctx, xp, L):
    nc = ctx.nc
    nc.vector.tensor_copy(xp[:, 0:7], xp[:, 14:7:-1])
    nc.vector.tensor_copy(xp[:, 7 + L:14 + L], xp[:, L + 5:L - 2:-1])


def build_nc():
    nc = bacc.Bacc("TRN2", target_bir_lowering=False, debug=False,
                   num_devices=N_CORES)
    x_ap = nc.dram_tensor("x", [ROWS_PER_CORE, L0], F16, kind="ExternalInput").ap()
    y_ap = nc.dram_tensor("y", [4, ROWS_PER_CORE, L0], F16, kind="ExternalOutput").ap()

    with tile.TileContext(nc) as tc:
        with tc.tile_pool(name="bufs", bufs=1) as pool, \
             tc.tile_pool(name="co", bufs=2) as copool, \
             tc.tile_pool(name="ob", bufs=2) as obpool, \
             tc.tile_pool(name="ps", bufs=8, space="PSUM") as pspool:
            ctx = Ctx(nc, pool, obpool, pspool)
            ctx.build_consts()

            for t in range(TILES_PER_CORE):
                rows = slice(t * 128, (t + 1) * 128)

                xp = copool.tile([128, L0 + 14], F16, tag="xp")
                nc.sync.dma_start(xp[:, 7:7 + L0], x_ap[rows, :])
                _emit_reflect(ctx, xp, L0)
                # deinterleave x on Act
                xe = pool.tile([128, 4103], F16, tag="xe")
                xo = pool.tile([128, 4103], F16, tag="xo")
                nc.scalar.copy(xe[:], xp[:, 0:8206:2])
                nc.scalar.copy(xo[:], xp[:, 1:8206:2])

                # level 1 on DVE (f16 mul/add fast modes), d1 and a1 interleaved
                d1 = copool.tile([128, L1], F16, tag="d1")
                a1p = pool.tile([128, L1 + 14], F16, tag="a1p")
                tA = pool.tile([128, L1], F16, tag="tA")
                tB = pool.tile([128, L1], F16, tag="tB")
                _interleave(
                    _ana_f16_thunks(ctx, xe, xo, d1[:], DEC_HI, L1, tA),
                    _ana_f16_thunks(ctx, xe, xo, a1p[:, 7:7 + L1], DEC_LO, L1, tB))
                _emit_reflect(ctx, a1p, L1)

                # band 3 on PE as early as possible (needs only d1)
                ob3 = obpool.tile([128, L0], F16, tag="ob")
                _emit_synth_pe(ctx, d1,
                               ob3, [(p, _synth_phase_taps(REC_HI, p)) for p in (0, 1)],
                               L0, 2)
                nc.sync.dma_start(y_ap[3, rows, :], ob3[:])

                # level 2: deinterleave a1p on Pool, then d2/a2 on DVE
                a1e = pool.tile([128, 2057], F16, tag="a1e")
                a1o = pool.tile([128, 2057], F16, tag="a1o")
                nc.gpsimd.tensor_copy(a1e[:], a1p[:, 0:4114:2])
                nc.gpsimd.tensor_copy(a1o[:], a1p[:, 1:4114:2])
                d2 = copool.tile([128, L2], F16, tag="d2")
                a2p = pool.tile([128, L2 + 14], F16, tag="a2p")
                _interleave(
                    _ana_f16_thunks(ctx, a1e, a1o, d2[:], DEC_HI, L2, tA),
                    _ana_f16_thunks(ctx, a1e, a1o, a2p[:, 7:7 + L2], DEC_LO, L2, tB))
                _emit_reflect(ctx, a2p, L2)

                # band 2 on PE: fused hi-then-lo U4 composite from d2
                ob2 = obpool.tile([128, L0], F16, tag="ob")
                _emit_synth_pe(ctx, d2, ob2, list(TAPS_HL.items()), L0, 4)
                nc.sync.dma_start(y_ap[2, rows, :], ob2[:])

                # level 3: d3 on DVE (needs a2 deint on Pool), a3 on Pool
                a2e = pool.tile([128, 1034], F16, tag="a2e")
                a2o = pool.tile([128, 1034], F16, tag="a2o")
                nc.gpsimd.tensor_copy(a2e[:], a2p[:, 0:2068:2])
                nc.gpsimd.tensor_copy(a2o[:], a2p[:, 1:2068:2])
                d3 = copool.tile([128, L3], F16, tag="d3")
                _interleave(_ana_f16_thunks(ctx, a2e, a2o, d3[:], DEC_HI, L3, tA))
                a3 = copool.tile([128, L3], F16, tag="a3")
                _emit_pool_ana(ctx, a2p, a3[:], DEC_LO, L3)

                # band 1: s1 (REC_HI) from d3 on PE -> v, then fused lo-lo U4
                v = copool.tile([128, L2], F16, tag="v")
                _emit_synth_pe(ctx, d3, v, [(p, _synth_phase_taps(REC_HI, p)) for p in (0, 1)],
                               L2, 2)
                ob1 = obpool.tile([128, L0], F16, tag="ob")
                _emit_synth_pe(ctx, v, ob1, list(TAPS_LL.items()), L0, 4)
                nc.sync.dma_start(y_ap[1, rows, :], ob1[:])

                # band 0: s1 (REC_LO) from a3 on PE -> u, then fused lo-lo U4
                u = copool.tile([128, L2], F16, tag="u")
                _emit_synth_pe(ctx, a3, u, [(p, _synth_phase_taps(REC_LO, p)) for p in (0, 1)],
                               L2, 2)
                ob0 = obpool.tile([128, L0], F16, tag="ob")
                _emit_synth_pe(ctx, u, ob0, list(TAPS_LL.items()), L0, 4)
                nc.sync.dma_start(y_ap[0, rows, :], ob0[:])

    nc.compile()
    return nc


_NC = None


def _get_nc():
    global _NC
    if _NC is None:
        _NC = build_nc()
    return _NC


def shard_inputs(x):
    rows = np.ascontiguousarray(x.reshape(-1, L0)).astype(np.float16)
    return [{"x": rows[c * ROWS_PER_CORE:(c + 1) * ROWS_PER_CORE]}
            for c in range(N_CORES)]


def unshard_outputs(results):
    out = np.empty((4, N_CORES * ROWS_PER_CORE, L0), np.float32)
    for c, r in enumerate(results):
        out[:, c * ROWS_PER_CORE:(c + 1) * ROWS_PER_CORE, :] = r["y"].astype(np.float32)
    return out.reshape(4, 16, 128, L0)


def kernel(x):
    x = np.asarray(x, np.float32)
    assert x.shape == (16, 128, L0), x.shape
    nc = _get_nc()
    res = run_bass_kernel_spmd(nc, shard_inputs(x), core_ids=list(range(N_CORES)))
    return unshard_outputs(res.results)
